# revision 30
# baseline (speedup 1.0000x reference)
"""Trainium2 Bass kernel for a BertPooler-style segment-reduce:

    first = h[:, 0, :]
    subj  = mean(h[b, subj_range[b,0]:subj_range[b,1], :])
    obj   = mean(h[b, obj_range[b,0]:obj_range[b,1], :])
    out   = tanh(concat([first, subj, obj]) @ W.T + b)

Strategy (8 NeuronCores, 4x2 grid: batch-groups x output-column-groups):
  - Core (bg, jg) owns 16 of the 64 batch rows and 512 of the 1024 output
    columns; W is sharded by output column so each core reads half of W.
  - Everything big rides in bf16 (tolerance is 2e-2): hidden state is cast
    on the host, W is cast + pre-arranged on the host so each W chunk DMA
    reads 6KB contiguous per partition.
  - Ranges cover <= 32 tokens; gather indices (start + lane offsets) and
    the 1/len reduction masks are host-precomputed (O(B) metadata, like
    the range packing itself), so the device does: load idx -> NB
    indirect gathers, each partition pulling TPP consecutive token rows
    in one contiguous DMA read (K_TPP env: 1/2/4).
  - Window sums/means are masked matmuls on the TensorEngine producing
    the transposed feature matrix featsT[3072, BL] directly; CLS rows are
    a strided DMA + identity-mask matmuls.
  - The pooler matmul streams the W shard through the PE as the moving
    bf16 operand; bias is a K=1 accumulating f32 matmul; tanh on ACT.
  - W chunk 0 streams concurrently with the gathers to keep the DMA
    engines saturated; chunks 1-3 are explicitly ordered after the last
    gather chunk (manual dep) so the gathers -- whose consumers are the
    long reduction tail -- are not starved by queue round-robin.
"""

import numpy as np

B, S, H = 64, 512, 1024
N_CORES = 8
GJ = 2                     # output-column groups (W shard factor)
GB = N_CORES // GJ         # batch groups
BL = B // GB               # batches per core
NG = BL // 4               # 4-batch gather groups per core
NGT = 2 * NG               # gather index columns (groups x {subj, obj})
JW = H // GJ               # output columns per core
K3 = 3 * H                 # feats dim
P = 128
NKT = K3 // P              # 24 contraction tiles
NWC = 4                    # W chunks
KTC = NKT // NWC           # k-tiles per W chunk
import os as _os
WMAX = 32                  # max range length the fast path supports
NW = 2 * BL                # windows per core (batches x {subj, obj})
# gather layout: TPP consecutive tokens per partition (one 2*TPP KB
# contiguous DRAM read per descriptor); a window spans PW partitions
TPP = int(_os.environ.get("K_TPP", "4"))
PW = WMAX // TPP           # partitions per window
WB = P // PW               # windows per 128-partition gather block
NB = NW // WB              # gather instruction count (>= 2)
# mask tensor layout (bf16): iB [0:16] | NB*TPP window mask blocks of
# width WB | bias row0 [JW] | ones row0 [BL]
MSK_B0 = BL + NB * TPP * WB       # bias column offset
MSK_O0 = MSK_B0 + JW              # ones column offset
MSKW = MSK_O0 + BL

_cache: dict = {}


def _consts_cpk():
    """Static part of the mask tensor: identity block + ones row."""
    import ml_dtypes
    msk = np.zeros((P, MSKW), ml_dtypes.bfloat16)
    msk[0:BL, 0:BL] = np.eye(BL, dtype=np.float32)
    msk[0, MSK_O0:MSK_O0 + BL] = 1.0
    return msk


def _build_fast(reps=1, hw_loop=False, num_devices=None):
    import contextlib
    import concourse.bass as bass
    import concourse.tile as tile
    from concourse import bacc, mybir

    f32 = mybir.dt.float32
    bf16 = mybir.dt.bfloat16
    i32 = mybir.dt.int32

    nc = bacc.Bacc("TRN2", target_bir_lowering=False, debug=False,
                   num_devices=num_devices or N_CORES)

    h = nc.dram_tensor("h", [BL * S, H], bf16, kind="ExternalInput")
    idxd = nc.dram_tensor("idx", [P, NB], i32, kind="ExternalInput")
    mskd = nc.dram_tensor("msk", [P, MSKW], bf16, kind="ExternalInput")
    wpk = nc.dram_tensor("wpk", [P, NKT * JW], bf16, kind="ExternalInput")
    out = nc.dram_tensor("out", [BL, JW], bf16, kind="ExternalOutput")

    with tile.TileContext(nc) as tc:
        with (
            tc.tile_pool(name="work", bufs=1) as wpool,
            tc.tile_pool(name="wtiles", bufs=NWC) as wtpool,
            tc.tile_pool(name="psum", bufs=1, space="PSUM") as ppool,
        ):
          loop_ctx = (tc.For_i(0, reps, 1) if hw_loop
                      else contextlib.nullcontext())
          with loop_ctx:
            for _rep in range(1 if hw_loop else reps):
                # --- tiny prefix loads; idx first (gathers wait on it) ---
                idx_t = wpool.tile([P, NB], i32, tag="idx")
                nc.sync.dma_start(idx_t[:], idxd[:, :])
                msk_t = wpool.tile([P, MSKW], bf16, tag="msk")
                nc.sync.dma_start(msk_t[:], mskd[:, :])

                # CLS rows: h[b*S, :] -- fixed-stride DMA on the ACT ring
                # (behind the ~1.3us Tanh table load; not critical-path)
                cls_t = wpool.tile([BL, H], bf16, tag="cls")
                h_bsd = h.ap().rearrange("(b s) d -> b s d", s=S)
                nc.scalar.dma_start(cls_t[:], h_bsd[:, 0, :])

                # --- W chunk 0 streams alongside the gathers ---
                wcs = [wtpool.tile([P, KTC * JW], bf16, tag="wc",
                                   name=f"wc{i}") for i in range(NWC)]
                nc.sync.dma_start(wcs[0][:], wpk[:, 0:KTC * JW])

                # --- the token windows: NB indirect DMAs; each partition
                # pulls TPP consecutive token rows in one contiguous read
                gt = wpool.tile([P, NB * TPP * H], bf16, tag="gt")
                gdmas = []
                for nb in range(NB):
                    gd = nc.gpsimd.indirect_dma_start(
                        out=gt[:, nb * TPP * H:(nb + 1) * TPP * H],
                        out_offset=None,
                        in_=h.ap(),
                        in_offset=bass.IndirectOffsetOnAxis(
                            ap=idx_t[:, nb:nb + 1], axis=0),
                        bounds_check=BL * S - 1,
                        oob_is_err=False,
                    )
                    gdmas.append(gd)

                # --- W chunks 1-3 staggered behind gather chunks 2-4 so
                # the gathers keep >=50% of HBM bandwidth while W streams
                for c4 in range(1, NWC):
                    eng = nc.sync if c4 % 2 == 0 else nc.scalar
                    wd = eng.dma_start(wcs[c4][:],
                                       wpk[:, c4 * KTC * JW:
                                           (c4 + 1) * KTC * JW])
                    if not _os.environ.get("K_NODEP"):
                        gi = min(c4 * NB // NWC, NB - 1)
                        bass._add_dep_helper(wd.ins, gdmas[gi].ins,
                                             sync=True,
                                             reason="stagger W after gathers")

                # --- reduction matmuls -> featsT[3072, BL] in PSUM ---
                # one PSUM tile per segment so each segment's SBUF copy only
                # waits on its own reductions; gather col c covers range
                # s = c // NG (subj, obj), batch group g = c % NG
                fps = [ppool.tile([P, 8 * BL], f32, tag=f"fp{s}",
                                  space="PSUM", name=f"fp{s}")
                       for s in range(3)]
                for ks in range(8):
                    nc.tensor.matmul(out=fps[0][:, ks * BL:(ks + 1) * BL],
                                     lhsT=cls_t[:BL, ks * P:(ks + 1) * P],
                                     rhs=msk_t[0:BL, 0:BL],
                                     start=True, stop=True)
                for nb in range(NB):
                    sg = nb * WB // BL            # block's range (0/1)
                    boff = nb * WB % BL           # first batch in block
                    for ks in range(8):
                        col = ks * BL + boff
                        for par in range(TPP):
                            m0 = BL + (nb * TPP + par) * WB
                            nc.tensor.matmul(
                                out=fps[1 + sg][:, col:col + WB],
                                lhsT=gt[:, (nb * TPP + par) * H + ks * P:
                                         (nb * TPP + par) * H + (ks + 1) * P],
                                rhs=msk_t[:, m0:m0 + WB],
                                start=(par == 0), stop=(par == TPP - 1))
                ft_sb = wpool.tile([P, NKT * BL], bf16, tag="ftsb")
                # per-segment copies so the pooler matmul can start on
                # segment 0 (CLS) while subj/obj reductions are in flight
                for seg in range(3):
                    nc.vector.tensor_copy(
                        ft_sb[:, seg * 8 * BL:(seg + 1) * 8 * BL],
                        fps[seg][:, :])

                # --- pooler matmul: out[BL, JW] = featsT.T @ Wshard + b ---
                # bias rides first (K=1, operands ready early) so the last
                # W chunk's matmuls are the final PSUM writes
                op_t = ppool.tile([BL, JW], f32, tag="op", space="PSUM")
                nc.tensor.matmul(out=op_t[:BL, :],
                                 lhsT=msk_t[0:1, MSK_O0:MSK_O0 + BL],
                                 rhs=msk_t[0:1, MSK_B0:MSK_B0 + JW],
                                 start=True, stop=False)
                for kt in range(NKT):
                    c4, t = divmod(kt, KTC)
                    nc.tensor.matmul(
                        out=op_t[:BL, :],
                        lhsT=ft_sb[:, kt * BL:(kt + 1) * BL],
                        rhs=wcs[c4][:, t * JW:(t + 1) * JW],
                        start=False, stop=(kt == NKT - 1))
                o_sb = wpool.tile([BL, JW], bf16, tag="osb")
                nc.scalar.activation(
                    out=o_sb[:BL, :], in_=op_t[:BL, :],
                    func=mybir.ActivationFunctionType.Tanh)
                nc.sync.dma_start(out[:, :], o_sb[:])

    nc.compile()
    return nc


def _get_nc():
    if "nc" not in _cache:
        _cache["nc"] = _build_fast()
    return _cache["nc"]


def _core_inputs(hidden_states, subj, obj, wt_full, bias_full, consts, c):
    """Build the in_map for core c = bg * GJ + jg."""
    import ml_dtypes
    bg, jg = divmod(c, GJ)
    lo = bg * BL

    # gather indices + window masks: block nb holds windows
    # w = nb*WB + p//PW (w: s = w//BL range, b = w%BL batch); partition p
    # reads TPP consecutive tokens from start_b + TPP*(p%PW)
    wloc = np.arange(P) // PW         # window-within-block per partition
    pp = np.arange(P) % PW            # partition-within-window
    idx = np.empty((P, NB), np.int32)
    msk = np.asarray(consts).copy()
    ranges = (subj, obj)
    for nb in range(NB):
        w = nb * WB + wloc
        s, b = w // BL, w % BL
        start = np.asarray([ranges[si][lo + bi, 0] for si, bi in zip(s, b)],
                           dtype=np.int64)
        length = np.asarray([ranges[si][lo + bi, 1] - ranges[si][lo + bi, 0]
                             for si, bi in zip(s, b)], dtype=np.int64)
        raw = b * S + start + TPP * pp
        idx[:, nb] = np.minimum(raw, BL * S - TPP).astype(np.int32)
        for par in range(TPP):
            j = TPP * pp + par
            wgt = (j < length) / np.maximum(length, 1)
            m = np.zeros((P, WB), np.float32)
            m[np.arange(P), wloc] = wgt
            c0 = BL + (nb * TPP + par) * WB
            msk[:, c0:c0 + WB] = m

    msk[0, MSK_B0:MSK_B0 + JW] = (
        np.asarray(bias_full).reshape(-1)[jg * JW:(jg + 1) * JW])

    # W shard pre-arranged: wpk[p, kt*JW + j] = W.T[kt*128 + p, jg*JW + j]
    key = ("wpk", jg)
    if key not in _cache:
        wsh = np.asarray(wt_full)[:, jg * JW:(jg + 1) * JW]
        wpk = np.ascontiguousarray(
            wsh.reshape(NKT, P, JW).transpose(1, 0, 2).reshape(P, NKT * JW)
        ).astype(ml_dtypes.bfloat16)
        _cache[key] = wpk

    key_h = ("h", bg)
    if key_h not in _cache:
        _cache[key_h] = np.ascontiguousarray(
            hidden_states[lo:lo + BL].reshape(BL * S, H)
        ).astype(ml_dtypes.bfloat16)
    return {
        "h": _cache[key_h],
        "idx": idx,
        "msk": msk,
        "wpk": _cache[key],
    }


def kernel(hidden_states, subj_range, obj_range, W, b):
    from concourse.bass_utils import run_bass_kernel_spmd

    hidden_states = np.asarray(hidden_states, dtype=np.float32)
    subj = np.asarray(subj_range).astype(np.int64)
    obj = np.asarray(obj_range).astype(np.int64)
    W = np.asarray(W, dtype=np.float32)
    b = np.asarray(b, dtype=np.float32)
    assert hidden_states.shape == (B, S, H)
    assert subj.shape == (B, 2) and obj.shape == (B, 2)

    max_len = max((subj[:, 1] - subj[:, 0]).max(), (obj[:, 1] - obj[:, 0]).max())
    assert max_len <= WMAX, "fast path requires range length <= 32"

    # per-invocation caches (inputs may differ between calls)
    for k in [k for k in _cache if isinstance(k, tuple)]:
        del _cache[k]

    nc = _get_nc()
    consts = _consts_cpk()
    wt_full = np.ascontiguousarray(W.T)            # [3072, 1024]
    bias_full = np.ascontiguousarray(b[None, :])   # [1, 1024]

    in_maps = [_core_inputs(hidden_states, subj, obj, wt_full, bias_full,
                            consts, c) for c in range(N_CORES)]

    res = run_bass_kernel_spmd(nc, in_maps, core_ids=list(range(N_CORES)))
    out = np.empty((B, H), np.float32)
    for c in range(N_CORES):
        bg, jg = divmod(c, GJ)
        out[bg * BL:(bg + 1) * BL, jg * JW:(jg + 1) * JW] = (
            res.results[c]["out"].astype(np.float32))
    return out
